# revision 55
# baseline (speedup 1.0000x reference)
"""Trainium2 Bass kernel for nn_Attention (dense transformer attention block).

Full-input contract: kernel(**inputs) takes the unsharded inputs and returns
the full output. 8 NeuronCores: tensor-parallel over head groups (4 heads) x
data-parallel over batch (2); core c = b*4 + g. Per core: q/k/v projections
for its head group, RoPE, causal flash-style attention (transposed-P layout,
softmax without max-subtraction), partial o_proj with its rows of Wo; the 4
partials per batch element are summed on the host (the all-reduce of the
row-sharded o_proj).

Round structure, all emitted as interleaved unit streams so the PE never
idles on the softmax chain:
  P(0); B(0)|P(1); B(1)|P(2)+O(0); B(2)|P(3)+O(1); B(3)|O(2); O(3)
P(sc) = V/Q/K projection for one 512-seq chunk (single-PSUM-bank passes so
evictions overlap the next pass), B(qc) = causal attention for one q chunk
(diagonal blocks restricted to their causally valid columns), O(qc) =
o_proj. Engine split: PE matmuls, ACT exp only, DVE everything else
(fast-approx reciprocal for the softmax denominator), GPSIMD den broadcast.

PSUM banks: b0/b1 projections, b2 den, b3 o_proj (last round also rotates
through b0/b1/b4), b4/b5 scores, b6/b7 ctx (alternating per head).
Host pre-tiles xt/weights so every DMA line is >=4KB contiguous.
"""
import contextlib
import numpy as np
import concourse.bass as bass
from concourse import bacc
import concourse.mybir as mybir
import concourse.tile as tile
from concourse.bass_utils import run_bass_kernel_spmd

F32 = mybir.dt.float32
F32R = mybir.dt.float32r
F16 = mybir.dt.float16
BF16 = mybir.dt.bfloat16
EXP = mybir.ActivationFunctionType.Exp
MMDT = {"f32r": F32R, "f16": F16, "bf16": BF16}

S = 2048
HID = 2048
D = 128
GH = 4            # heads per core
GW = GH * D       # 512
NCORES = 8
SC = S // 512     # 4 column chunks
HC = HID // 128   # 16 contraction chunks
SCALE = float(D) ** -0.5
NEG = -1.0e30

DTYPE = "f16"     # matmul dtype: 'f16' | 'bf16' | 'f32r'


def _build(variant, dt):
    MDT = MMDT[dt]
    two_byte = dt in ("f16", "bf16")
    IDT = MDT if two_byte else F32
    nc = bacc.Bacc("TRN2", target_bir_lowering=False, debug=False,
                   num_devices=NCORES)
    # host-pre-tiled layouts: partition dim first, >=4KB contiguous lines
    xt = nc.dram_tensor("xt", [128, SC, HC // 4, 4, 512], IDT,
                        kind="ExternalInput").ap()
    # sc=0 duplicate in v-block-major layout: first V pass depends on 1/4 of
    # the chunk instead of all of it
    xt0 = nc.dram_tensor("xt0", [128, 4, HC // 4, 4, 128], IDT,
                         kind="ExternalInput").ap()
    wq = nc.dram_tensor("wq", [128, HC, GW], IDT, kind="ExternalInput").ap()
    wk = nc.dram_tensor("wk", [128, HC, GW], IDT, kind="ExternalInput").ap()
    wv = nc.dram_tensor("wv", [128, HC, GW], IDT, kind="ExternalInput").ap()
    wo = nc.dram_tensor("wo", [128, GH, HID], IDT, kind="ExternalInput").ap()
    cost = nc.dram_tensor("cost", [D, S], IDT, kind="ExternalInput").ap()
    sint = nc.dram_tensor("sint", [D, S], IDT, kind="ExternalInput").ap()
    btpl = nc.dram_tensor("btpl", [D, 896], F32, kind="ExternalInput").ap()
    out = nc.dram_tensor("out", [S, HID], IDT, kind="ExternalOutput").ap()

    def _bc(ap):
        return ap if two_byte else ap.bitcast(F32R)

    xt_r = _bc(xt)
    xt0_r = _bc(xt0)
    wq_r = _bc(wq)
    wk_r = _bc(wk)
    wv_r = _bc(wv)
    wo_r = _bc(wo)

    XB = 4                   # h-chunks per xt/weight DMA
    NXT = HC // XB           # 4 tiles per s-chunk

    with tile.TileContext(nc) as tc:
        with contextlib.ExitStack() as ctx:
            persist = ctx.enter_context(tc.tile_pool(name="persist", bufs=1))
            psum = ctx.enter_context(tc.tile_pool(name="psum", bufs=1, space="PSUM"))
            work = ctx.enter_context(tc.tile_pool(name="work", bufs=1))

            _n = [0]

            def bank(i, shape=(128, 512)):
                _n[0] += 1
                return psum.tile(list(shape), F32, tag=f"b{i}", name=f"bk{i}_{_n[0]}")

            qts = [[persist.tile([128, 512], MDT, tag=f"qt{h}_{s}",
                                 name=f"qt{h}_{s}") for s in range(SC)]
                   for h in range(GH)]
            kts = [[persist.tile([128, 512], MDT, tag=f"kt{h}_{s}",
                                 name=f"kt{h}_{s}") for s in range(SC)]
                   for h in range(GH)]
            vts = [persist.tile([128, GW], MDT, tag=f"v{st}", name=f"v{st}")
                   for st in range(HC)]
            cos_sb = persist.tile([128, S], MDT, tag="cos")
            sin_sb = persist.tile([128, S], MDT, tag="sin")
            btpl_sb = persist.tile([128, 896], F32, tag="btpl")
            ones_f = persist.tile([128, 1], F32, tag="onesf")
            ones = persist.tile([128, 1], MDT, tag="ones")
            wo_sb = persist.tile([128, GH, HID], MDT, tag="wo")
            wq_sb = persist.tile([128, HC, GW], MDT, tag="wq")
            wk_sb = persist.tile([128, HC, GW], MDT, tag="wk")
            wv_sb = persist.tile([128, HC, GW], MDT, tag="wv")

            # wv is needed by the very first MMs; fine-grained (2-h) DMAs
            # spread it across many queues so the first pass starts early.
            # The rest of the weights stream in behind it, interleaved into
            # P(0)'s unit stream.
            for j in range(HC // 2):
                jsl = slice(j * 2, (j + 1) * 2)
                nc.sync.dma_start(out=wv_sb[:, jsl, :], in_=wv_r[:, jsl, :])
            nc.vector.memset(ones_f, 1.0)
            nc.vector.tensor_copy(ones, ones_f)
            ebias = persist.tile([128, 1], F32, tag="ebias")
            nc.vector.memset(ebias, -4.0)

            def w_units():
                units = []

                def wchunk(w_sb, w_r, j):
                    jsl = slice(j * XB, (j + 1) * XB)
                    nc.sync.dma_start(out=w_sb[:, jsl, :], in_=w_r[:, jsl, :])

                for j in range(NXT):
                    units.append(lambda j=j: wchunk(wq_sb, wq_r, j))
                units.append(lambda: (nc.sync.dma_start(out=cos_sb, in_=cost),
                                      nc.sync.dma_start(out=sin_sb, in_=sint)))
                for j in range(NXT):
                    units.append(lambda j=j: wchunk(wk_sb, wk_r, j))
                units.append(lambda: (nc.sync.dma_start(out=btpl_sb, in_=btpl),
                                      nc.sync.dma_start(out=wo_sb, in_=wo_r)))
                return units

            # ---- P(sc): V/Q/K projection for one s-chunk ----------------
            # 12 single-bank passes (V st0..3, Q d0..3, K d0..3) alternating
            # banks b0/b1; evictions (DVE) overlap the next pass's MMs.
            def p_units(sc):
                ssl = slice(sc * 512, (sc + 1) * 512)
                units = []
                st = {}

                def prep():
                    st['xt'] = []
                    for j in range(NXT):
                        t = work.tile([128, XB, 512], MDT, tag="xt", bufs=8,
                                      name=f"xt_{sc}_{j}")
                        if sc != 0:
                            nc.sync.dma_start(out=t, in_=xt_r[:, sc, j])
                        st['xt'].append(t)
                    if sc == 0:
                        # v-block-granular, v-major loads: V pass st0 only
                        # waits on the v=0 columns of the chunk. Issue from
                        # the idle scalar/vector queues so the ~0.6us
                        # per-DMA issue cost doesn't serialize behind wv.
                        for v in range(4):
                            for j in range(NXT):
                                nc.scalar.dma_start(
                                    out=st['xt'][j][:, :, v * 128:(v + 1) * 128],
                                    in_=xt0_r[:, v, j])

                units.append(prep)
                pidx = [0]

                def v_pass(v):
                    b = [None]

                    def mms(k, v=v):
                        if k == 0:
                            b[0] = bank(pidx[0] % 2)
                            pidx[0] += 1
                        for h in (2 * k, 2 * k + 1):
                            nc.tensor.matmul(
                                b[0],
                                st['xt'][h // XB][:, h % XB, v * 128:(v + 1) * 128],
                                wv_sb[:, h, :],
                                start=(h == 0), stop=(h == HC - 1))

                    for k in range(HC // 2):
                        units.append(lambda k=k, mms=mms: mms(k))

                    def evict(v=v):
                        nc.vector.tensor_copy(vts[sc * 4 + v], b[0])

                    units.append(evict)

                def qk_pass(w_sb, dst, d):
                    b = [None]

                    def mms(k, d=d):
                        if k == 0:
                            b[0] = bank(pidx[0] % 2)
                            pidx[0] += 1
                        for h in (2 * k, 2 * k + 1):
                            nc.tensor.matmul(
                                b[0], w_sb[:, h, d * 128:(d + 1) * 128],
                                st['xt'][h // XB][:, h % XB, :],
                                start=(h == 0), stop=(h == HC - 1))

                    for k in range(HC // 2):
                        units.append(lambda k=k, mms=mms: mms(k))

                    def evict(dst=dst):
                        # single PSUM read (copy), then RoPE in f16 SBUF —
                        # extra PSUM reads backpressure the PE drain port
                        t2 = work.tile([128, 512], MDT, tag="t2", bufs=3,
                                       name=f"t2_{sc}_{dst.tensor.name}")
                        nc.vector.tensor_copy(t2, b[0])     # frees the bank
                        t1 = work.tile([128, 512], MDT, tag="t1", bufs=3,
                                       name=f"t1_{sc}_{dst.tensor.name}")
                        nc.vector.tensor_mul(t1, t2, cos_sb[:, ssl])
                        nc.vector.tensor_mul(dst[0:64, :], t2[64:128, :],
                                             sin_sb[64:128, ssl])
                        nc.vector.tensor_mul(dst[64:128, :], t2[0:64, :],
                                             sin_sb[0:64, ssl])
                        nc.vector.tensor_add(dst, dst, t1)

                    units.append(evict)

                for v in range(4):
                    v_pass(v)
                for d in range(GH):
                    qk_pass(wq_sb, qts[d][sc], d)
                for d in range(GH):
                    qk_pass(wk_sb, kts[d][sc], d)
                return units

            # ---- B(qc): causal attention for one q chunk ---------------
            ctx_t = {}
            sps_idx = [0]
            head_idx = [0]

            def b_units(qc):
                if variant == "causal":
                    # diagonal blocks (restricted to valid columns) spread
                    # among the full blocks so their mask-add chains overlap;
                    # the first block must be full-width (j=0)
                    diag = [(4 * qc + j, 128 * j) for j in range(4)]
                    full = [(kb, 0) for kb in range(4 * qc)]
                    order = [diag[0]]
                    di, fi = 1, 0
                    while di < 4 or fi < len(full):
                        if fi < len(full):
                            order.append(full[fi])
                            fi += 1
                        if di < 4:
                            order.append(diag[di])
                            di += 1
                else:
                    order = [(kb, 0) for kb in range(HC)]
                units = []
                ctx_t[qc] = []
                for hd in range(GH):
                    st = {}

                    def start_head(st=st):
                        st['ctxps'] = bank(6 + head_idx[0] % 2)
                        head_idx[0] += 1
                        st['dacc'] = work.tile([128, 512], MDT, tag="dacc",
                                               bufs=2, name=f"dacc_{qc}_{hd}")
                        st['pend'] = None

                    def flush(last, st=st, hd=hd):
                        pexp, first, kbp, off = st['pend']
                        n = 512 - off
                        nc.tensor.matmul(st['ctxps'][:, off:512],
                                         vts[kbp][:, hd * 128:(hd + 1) * 128],
                                         pexp[:, 0:n], start=first, stop=last)

                    def kb_iter(i, kb, off, st=st, hd=hd,
                                start_head=start_head, flush=flush):
                        if i == 0:
                            start_head()
                        n = 512 - off
                        sps = bank(4 + sps_idx[0] % 2)
                        sps_idx[0] += 1
                        nc.tensor.matmul(
                            sps[:, off:512],
                            kts[hd][kb // 4][:, (kb % 4) * 128:(kb % 4 + 1) * 128],
                            qts[hd][qc][:, off:512], start=True, stop=True)
                        if variant == "causal" and kb >= 4 * qc:
                            # triangular mask on the 128-wide diagonal block
                            nc.vector.tensor_add(sps[:, off:off + 128],
                                                 sps[:, off:off + 128],
                                                 btpl_sb[:, 384:512])
                        pexp = work.tile([128, 512], MDT, tag="pexp", bufs=4,
                                         name=f"pexp_{qc}_{hd}_{kb}")
                        # bias -4 keeps the f16 denominator accumulator well
                        # below overflow; it cancels in ctx/den
                        nc.scalar.activation(pexp[:, 0:n], sps[:, off:512],
                                             EXP, scale=SCALE, bias=ebias)
                        # denominator partial sum (DVE) is pended one
                        # iteration: it depends on exp(kb), and emitting it
                        # now would block the next mask-add (and so the next
                        # exp) behind it in the DVE FIFO
                        if st['pend'] is not None:
                            flush(False)
                            dadd(st)
                        st['pend'] = (pexp, i == 0, kb, off)

                    def dadd(st):
                        pexp, first, kbp, off = st['pend']
                        n = 512 - off
                        if first:
                            nc.vector.tensor_copy(st['dacc'], pexp)
                        else:
                            nc.vector.tensor_add(st['dacc'][:, off:512],
                                                 st['dacc'][:, off:512],
                                                 pexp[:, 0:n])

                    def tail(st=st, hd=hd, flush=flush):
                        flush(True)
                        dadd(st)
                        denps = bank(2, shape=(1, 512))
                        nc.tensor.matmul(denps, ones, st['dacc'],
                                         start=True, stop=True)
                        rcp = work.tile([1, 512], F32, tag="rcp", bufs=2,
                                        name=f"rcp_{qc}_{hd}")
                        nc.vector.reciprocal_approx_fast(rcp, denps)
                        dbc = work.tile([128, 512], F32, tag="dbc", bufs=2,
                                        name=f"dbc_{qc}_{hd}")
                        nc.gpsimd.partition_broadcast(dbc, rcp)
                        ct = work.tile([128, 512], MDT, tag="ctx", bufs=8,
                                       name=f"ctx_{qc}_{hd}")
                        nc.vector.tensor_mul(ct, st['ctxps'], dbc)  # frees ctx
                        ctx_t[qc].append(ct)

                    for i, (kb, off) in enumerate(order):
                        units.append(lambda i=i, kb=kb, off=off,
                                     kb_iter=kb_iter: kb_iter(i, kb, off))
                    units.append(tail)
                return units

            # ---- O(qc): o_proj for one q chunk -------------------------
            def o_units(qc, banks, mode="split"):
                units = []
                oidx = [0]

                ots = {}

                def oevict(ops, qb, ob, cnt, last=False):
                    # one wide ot tile per q block; a single [128,2048] DMA
                    # per q block keeps the sync queue's ~0.6us-per-DMA
                    # issue cost off the critical path. The very last block
                    # DMAs per-ob so the kernel doesn't end waiting on one
                    # long transfer.
                    if qb not in ots:
                        ots[qb] = work.tile([128, HID], IDT, tag="outsb",
                                            bufs=2, name=f"ot_{qc}_{qb}")
                    ot = ots[qb]
                    if mode == "split" and cnt % 2 == 0:
                        nc.scalar.copy(ot[:, ob * 512:(ob + 1) * 512], ops)
                    else:
                        nc.vector.tensor_copy(ot[:, ob * 512:(ob + 1) * 512],
                                              ops)
                    rows = slice((qc * 4 + qb) * 128, (qc * 4 + qb + 1) * 128)
                    if last:
                        nc.sync.dma_start(
                            out=out[rows, ob * 512:(ob + 1) * 512],
                            in_=ot[:, ob * 512:(ob + 1) * 512])
                    elif ob == 3:
                        nc.sync.dma_start(out=out[rows, :], in_=ot)

                def oproj(qb, ob):
                    ops = bank(banks[oidx[0] % len(banks)])
                    oidx[0] += 1
                    for hd in range(GH):
                        nc.tensor.matmul(
                            ops, ctx_t[qc][hd][:, qb * 128:(qb + 1) * 128],
                            wo_sb[:, hd, ob * 512:(ob + 1) * 512],
                            start=(hd == 0), stop=(hd == GH - 1))
                    oevict(ops, qb, ob, oidx[0])

                if len(banks) >= 4:
                    # 4-bank sets: hd-major with ob inner — each ct stationary
                    # is loaded once and streams 4 matmuls (no LDW
                    # serialization); sets alternate so evicts overlap
                    def oproj4(qb):
                        bset = banks[4 * (qb % (len(banks) // 4)):]
                        opss = [bank(bset[ob]) for ob in range(4)]
                        for hd in range(GH):
                            for ob in range(4):
                                nc.tensor.matmul(
                                    opss[ob],
                                    ctx_t[qc][hd][:, qb * 128:(qb + 1) * 128],
                                    wo_sb[:, hd, ob * 512:(ob + 1) * 512],
                                    start=(hd == 0), stop=(hd == GH - 1))
                        for ob in range(4):
                            oevict(opss[ob], qb, ob, ob, last=(qb == 3))

                    for qb in range(4):
                        units.append(lambda qb=qb, oproj4=oproj4: oproj4(qb))
                else:
                    for qb in range(4):
                        for ob in range(4):
                            units.append(lambda qb=qb, ob=ob, oproj=oproj:
                                         oproj(qb, ob))
                return units

            # ---- emit: P(0) | weight loads, then B(qc) | P(qc+1)+O(qc-1)
            p0 = p_units(0)
            wu = w_units()
            wi = 0
            for i, u in enumerate(p0):
                u()
                tgt = min(len(wu), (i + 1) * len(wu) * 3 // len(p0))
                while wi < tgt:
                    wu[wi]()
                    wi += 1
            while wi < len(wu):
                wu[wi]()
                wi += 1
            for qc in range(SC):
                bu = b_units(qc)
                fill = p_units(qc + 1) if qc + 1 < SC else []
                if qc >= 1:
                    # spread O(qc-1) units evenly through the fillers;
                    # in the last round ACT is paced by exp, so evict on DVE
                    ou = o_units(qc - 1, [3],
                             mode="dve" if qc == SC - 1 else "split")
                    merged = []
                    no, nf = len(ou), len(fill)
                    if nf == 0:
                        merged = ou
                    else:
                        oi = 0
                        for i, f in enumerate(fill):
                            merged.append(f)
                            tgt = (i + 1) * no // nf
                            while oi < tgt:
                                merged.append(ou[oi])
                                oi += 1
                        merged.extend(ou[oi:])
                    fill = merged
                na, nb = len(fill), len(bu)
                # reserve a few filler units past the round boundary so the
                # next round's softmax pipeline fill is hidden
                res = min(8, na)
                ai = 0
                for i, u in enumerate(bu):
                    u()
                    tgt = (i + 1) * (na - res) // nb
                    while ai < tgt:
                        fill[ai]()
                        ai += 1
                while ai < na:
                    fill[ai]()
                    ai += 1
            for u in o_units(SC - 1, [3, 0, 1, 4, 5, 6, 7, 2]):
                u()
    nc.compile()
    return nc


_CACHE = {}


def _get(variant, dt=None):
    dt = dt or DTYPE
    if (variant, dt) not in _CACHE:
        _CACHE[(variant, dt)] = _build(variant, dt)
    return _CACHE[(variant, dt)]


def _rope_tables():
    inv = 1.0 / (10000.0 ** (np.arange(0, D, 2, dtype=np.float64) / D))  # [64]
    t = np.arange(S, dtype=np.float64)
    fr = np.outer(inv, t)                       # [64, S]
    cosT = np.concatenate([np.cos(fr), np.cos(fr)], 0).astype(np.float32)
    # partition-swapped sign-folded sin: rows 0:64 = +sin, rows 64:128 = -sin
    sinT = np.concatenate([np.sin(fr), -np.sin(fr)], 0).astype(np.float32)
    return cosT, sinT


def _btpl_causal():
    # additive mask template: NEG where k > c-384 else 0
    k = np.arange(128)[:, None]
    c = np.arange(896)[None, :]
    return np.where(k > c - 384, np.float32(NEG), np.float32(0.0)).astype(np.float32)


def _np_cast(a, dt):
    if dt == "f16":
        return a.astype(np.float16)
    if dt == "bf16":
        import ml_dtypes
        return a.astype(ml_dtypes.bfloat16)
    return a


def _numpy_fallback(hs, Wq, Wk, Wv, Wo, mask):
    B = hs.shape[0]
    cosT, sinT = _rope_tables()
    cos = cosT.T[None, :, None, :]
    sin = np.abs(sinT).T[None, :, None, :]
    outs = []
    for b in range(B):
        x = hs[b]
        q = (x @ Wq).reshape(S, 16, D)[None]
        k = (x @ Wk).reshape(S, 16, D)[None]
        vv = (x @ Wv).reshape(S, 16, D)

        def rope(z):
            z1, z2 = z[..., :64], z[..., 64:]
            rot = np.concatenate([-z2, z1], -1)
            return z * cos + rot * sin

        q, k = rope(q)[0], rope(k)[0]
        o = np.empty((S, 16, D), np.float32)
        m = mask[0, 0]
        for h in range(16):
            sc = (q[:, h] @ k[:, h].T) * SCALE
            sc = np.where(m == 0, -np.inf, sc)
            sc -= sc.max(-1, keepdims=True)
            p = np.exp(sc)
            p /= p.sum(-1, keepdims=True)
            o[:, h] = p @ vv[:, h]
        outs.append(o.reshape(S, HID) @ Wo)
    return np.stack(outs).astype(np.float32)


def _tile_xt(hsT, dt):
    # [2048 h, 2048 s] -> [128 p, 4 sc, 4 j, 4 hh, 512] with h = (4j+hh)*128+p
    a = _np_cast(hsT, dt).reshape(HC, 128, SC, 512)
    a = a.transpose(1, 2, 0, 3).reshape(128, SC, NXT_H, 4, 512)
    return np.ascontiguousarray(a)


def _tile_xt0(hsT, dt):
    # sc=0 slice in v-block-major layout: [128 p, 4 v, 4 j, 4 hh, 128]
    a = _np_cast(hsT[:, 0:512], dt).reshape(HC, 128, 4, 128)
    a = a.transpose(1, 2, 0, 3).reshape(128, 4, NXT_H, 4, 128)
    return np.ascontiguousarray(a)


NXT_H = HC // 4


def _tile_w(w, dt):
    # [2048 h, 512] -> [128 p, 16 c, 512] with h = c*128+p
    a = _np_cast(w, dt).reshape(HC, 128, GW).transpose(1, 0, 2)
    return np.ascontiguousarray(a)


def _tile_wo(w, dt):
    # [512 r, 2048] -> [128 p, 4 hd, 2048] with r = hd*128+p
    a = _np_cast(w, dt).reshape(GH, 128, HID).transpose(1, 0, 2)
    return np.ascontiguousarray(a)


def make_in_maps(inputs, variant):
    hs = np.asarray(inputs["hidden_states"], dtype=np.float32)
    Wq, Wk, Wv, Wo = (np.asarray(inputs[w], dtype=np.float32)
                      for w in ("Wq", "Wk", "Wv", "Wo"))
    cosT, sinT = _rope_tables()
    btpl = _btpl_causal() if variant == "causal" else np.zeros((128, 896), np.float32)

    in_maps = []
    for c in range(NCORES):
        b, g = divmod(c, GH)
        gsl = slice(g * GW, (g + 1) * GW)
        hsT = np.ascontiguousarray(hs[b].T)
        in_maps.append({
            "xt": _tile_xt(hsT, DTYPE),
            "xt0": _tile_xt0(hsT, DTYPE),
            "wq": _tile_w(Wq[:, gsl], DTYPE),
            "wk": _tile_w(Wk[:, gsl], DTYPE),
            "wv": _tile_w(Wv[:, gsl], DTYPE),
            "wo": _tile_wo(Wo[gsl, :], DTYPE),
            "cost": _np_cast(cosT, DTYPE), "sint": _np_cast(sinT, DTYPE),
            "btpl": btpl,
        })
    return in_maps


def kernel(hidden_states, Wq, Wk, Wv, Wo, attention_mask):
    hs = np.asarray(hidden_states, dtype=np.float32)
    Wq, Wk, Wv, Wo = (np.asarray(w, dtype=np.float32) for w in (Wq, Wk, Wv, Wo))
    mask = np.asarray(attention_mask)
    B = hs.shape[0]

    m3 = mask.reshape(-1, mask.shape[-2], mask.shape[-1])
    m2 = m3[0]
    same = all(np.array_equal(m2, m3[i]) for i in range(1, m3.shape[0]))
    if not same:
        return _numpy_fallback(hs, Wq, Wk, Wv, Wo, mask)
    if np.all(m2 == 1):
        variant = "full"
    elif np.array_equal(m2 != 0, np.tril(np.ones((S, S), dtype=bool))):
        variant = "causal"
    else:
        return _numpy_fallback(hs, Wq, Wk, Wv, Wo, mask)

    in_maps = make_in_maps(
        {"hidden_states": hs, "Wq": Wq, "Wk": Wk, "Wv": Wv, "Wo": Wo}, variant)

    nc = _get(variant)
    res = run_bass_kernel_spmd(nc, in_maps, list(range(NCORES))).results
    out = np.zeros((B, S, HID), np.float32)
    for c in range(NCORES):
        b = c // GH
        out[b] += res[c]["out"]
    return out


# revision 61
# speedup vs baseline: 1.2157x; 1.2157x over previous
"""Trainium2 Bass kernel for nn_Attention (dense transformer attention block).

Full-input contract: kernel(**inputs) takes the unsharded inputs and returns
the full output. 8 NeuronCores: tensor-parallel over head groups (4 heads) x
data-parallel over batch (2); core c = b*4 + g. Per core: q/k/v projections
for its head group, RoPE, causal flash-style attention (transposed-P layout,
softmax without max-subtraction), partial o_proj with its rows of Wo; the 4
partials per batch element are summed on the host (the all-reduce of the
row-sharded o_proj).

Round structure, all emitted as interleaved unit streams so the PE never
idles on the softmax chain:
  P(0); B(0)|P(1); B(1)|P(2)+O(0); B(2)|P(3)+O(1); B(3)|O(2); O(3)
P(sc) = V/Q/K projection for one 512-seq chunk (single-PSUM-bank passes so
evictions overlap the next pass), B(qc) = causal attention for one q chunk
(diagonal blocks restricted to their causally valid columns), O(qc) =
o_proj. Engine split: PE matmuls, ACT exp only, DVE everything else
(fast-approx reciprocal for the softmax denominator), GPSIMD den broadcast.

PSUM banks: b0/b1 projections, b2 den, b3 o_proj (last round also rotates
through b0/b1/b4), b4/b5 scores, b6/b7 ctx (alternating per head).
Host pre-tiles xt/weights so every DMA line is >=4KB contiguous.
"""
import contextlib
import numpy as np
import concourse.bass as bass
from concourse import bacc
import concourse.mybir as mybir
import concourse.tile as tile
from concourse.bass_utils import run_bass_kernel_spmd

F32 = mybir.dt.float32
F32R = mybir.dt.float32r
F16 = mybir.dt.float16
BF16 = mybir.dt.bfloat16
EXP = mybir.ActivationFunctionType.Exp
MMDT = {"f32r": F32R, "f16": F16, "bf16": BF16}

S = 2048
HID = 2048
D = 128
GH = 4            # heads per core
GW = GH * D       # 512
NCORES = 8
SC = S // 512     # 4 column chunks
HC = HID // 128   # 16 contraction chunks
SCALE = float(D) ** -0.5
NEG = -1.0e30

DTYPE = "f16"     # matmul dtype: 'f16' | 'bf16' | 'f32r'


def _build(variant, dt):
    MDT = MMDT[dt]
    two_byte = dt in ("f16", "bf16")
    IDT = MDT if two_byte else F32
    nc = bacc.Bacc("TRN2", target_bir_lowering=False, debug=False,
                   num_devices=NCORES)
    # host-pre-tiled layouts: partition dim first, >=4KB contiguous lines
    xt = nc.dram_tensor("xt", [128, SC, HC // 4, 4, 512], IDT,
                        kind="ExternalInput").ap()
    # sc=0 duplicate in v-block-major layout: first V pass depends on 1/4 of
    # the chunk instead of all of it
    xt0 = nc.dram_tensor("xt0", [128, 4, HC // 4, 4, 128], IDT,
                         kind="ExternalInput").ap()
    wq = nc.dram_tensor("wq", [128, HC, GW], IDT, kind="ExternalInput").ap()
    wk = nc.dram_tensor("wk", [128, HC, GW], IDT, kind="ExternalInput").ap()
    wv = nc.dram_tensor("wv", [128, HC, GW], IDT, kind="ExternalInput").ap()
    wo = nc.dram_tensor("wo", [128, GH, HID], IDT, kind="ExternalInput").ap()
    cost = nc.dram_tensor("cost", [D, S], IDT, kind="ExternalInput").ap()
    sint = nc.dram_tensor("sint", [D, S], IDT, kind="ExternalInput").ap()
    btpl = nc.dram_tensor("btpl", [D, 896], F32, kind="ExternalInput").ap()
    lindt = nc.dram_tensor("lindt", [128, 128], IDT, kind="ExternalInput").ap()
    negdt = nc.dram_tensor("negdt", [128, 128], IDT, kind="ExternalInput").ap()
    out = nc.dram_tensor("out", [S, HID], IDT, kind="ExternalOutput").ap()

    def _bc(ap):
        return ap if two_byte else ap.bitcast(F32R)

    xt_r = _bc(xt)
    xt0_r = _bc(xt0)
    wq_r = _bc(wq)
    wk_r = _bc(wk)
    wv_r = _bc(wv)
    wo_r = _bc(wo)

    XB = 4                   # h-chunks per xt/weight DMA
    NXT = HC // XB           # 4 tiles per s-chunk

    with tile.TileContext(nc) as tc:
        with contextlib.ExitStack() as ctx:
            persist = ctx.enter_context(tc.tile_pool(name="persist", bufs=1))
            psum = ctx.enter_context(tc.tile_pool(name="psum", bufs=1, space="PSUM"))
            work = ctx.enter_context(tc.tile_pool(name="work", bufs=1))

            _n = [0]

            def bank(i, shape=(128, 512)):
                _n[0] += 1
                return psum.tile(list(shape), F32, tag=f"b{i}", name=f"bk{i}_{_n[0]}")

            qts = [[persist.tile([128, 512], MDT, tag=f"qt{h}_{s}",
                                 name=f"qt{h}_{s}") for s in range(SC)]
                   for h in range(GH)]
            kts = [[persist.tile([128, 512], MDT, tag=f"kt{h}_{s}",
                                 name=f"kt{h}_{s}") for s in range(SC)]
                   for h in range(GH)]
            vts = [persist.tile([128, GW], MDT, tag=f"v{st}", name=f"v{st}")
                   for st in range(HC)]
            cos_sb = persist.tile([128, S], MDT, tag="cos")
            sin_sb = persist.tile([128, S], MDT, tag="sin")
            btpl_sb = persist.tile([128, 896], F32, tag="btpl")
            ones_f = persist.tile([128, 1], F32, tag="onesf")
            ones = persist.tile([128, 1], MDT, tag="ones")
            wo_sb = persist.tile([128, GH, HID], MDT, tag="wo")
            wq_sb = persist.tile([128, HC, GW], MDT, tag="wq")
            wk_sb = persist.tile([128, HC, GW], MDT, tag="wk")
            wv_sb = persist.tile([128, HC, GW], MDT, tag="wv")

            # wv is needed by the very first MMs; fine-grained (2-h) DMAs
            # spread it across many queues so the first pass starts early.
            # The rest of the weights stream in behind it, interleaved into
            # P(0)'s unit stream.
            for j in range(HC // 2):
                jsl = slice(j * 2, (j + 1) * 2)
                nc.sync.dma_start(out=wv_sb[:, jsl, :], in_=wv_r[:, jsl, :])
            nc.vector.memset(ones_f, 1.0)
            nc.vector.tensor_copy(ones, ones_f)
            ebias = persist.tile([128, 1], F32, tag="ebias")
            nc.vector.memset(ebias, -4.0)
            lind = persist.tile([128, 128], MDT, tag="lind")
            negd = persist.tile([128, 128], MDT, tag="negd")
            nc.sync.dma_start(out=lind, in_=_bc(lindt))
            nc.sync.dma_start(out=negd, in_=_bc(negdt))

            def w_units():
                units = []

                def wchunk(w_sb, w_r, j):
                    jsl = slice(j * XB, (j + 1) * XB)
                    nc.sync.dma_start(out=w_sb[:, jsl, :], in_=w_r[:, jsl, :])

                for j in range(NXT):
                    units.append(lambda j=j: wchunk(wq_sb, wq_r, j))
                units.append(lambda: (nc.sync.dma_start(out=cos_sb, in_=cost),
                                      nc.sync.dma_start(out=sin_sb, in_=sint)))
                for j in range(NXT):
                    units.append(lambda j=j: wchunk(wk_sb, wk_r, j))
                units.append(lambda: (nc.sync.dma_start(out=btpl_sb, in_=btpl),
                                      nc.sync.dma_start(out=wo_sb, in_=wo_r)))
                return units

            # ---- P(sc): V/Q/K projection for one s-chunk ----------------
            # 12 single-bank passes (V st0..3, Q d0..3, K d0..3) alternating
            # banks b0/b1; evictions (DVE) overlap the next pass's MMs.
            def p_units(sc):
                ssl = slice(sc * 512, (sc + 1) * 512)
                units = []
                st = {}

                def prep():
                    st['xt'] = []
                    for j in range(NXT):
                        t = work.tile([128, XB, 512], MDT, tag="xt", bufs=8,
                                      name=f"xt_{sc}_{j}")
                        if sc != 0:
                            nc.sync.dma_start(out=t, in_=xt_r[:, sc, j])
                        st['xt'].append(t)
                    if sc == 0:
                        # v-block-granular, v-major loads: V pass st0 only
                        # waits on the v=0 columns of the chunk. Issue from
                        # the idle scalar/vector queues so the ~0.6us
                        # per-DMA issue cost doesn't serialize behind wv.
                        for v in range(4):
                            for j in range(NXT):
                                nc.scalar.dma_start(
                                    out=st['xt'][j][:, :, v * 128:(v + 1) * 128],
                                    in_=xt0_r[:, v, j])

                units.append(prep)
                pidx = [0]

                def v_pass(v):
                    b = [None]

                    def mms(k, v=v):
                        if k == 0:
                            b[0] = bank(pidx[0] % 2)
                            pidx[0] += 1
                        for h in (2 * k, 2 * k + 1):
                            nc.tensor.matmul(
                                b[0],
                                st['xt'][h // XB][:, h % XB, v * 128:(v + 1) * 128],
                                wv_sb[:, h, :],
                                start=(h == 0), stop=(h == HC - 1))

                    for k in range(HC // 2):
                        units.append(lambda k=k, mms=mms: mms(k))

                    def evict(v=v):
                        nc.vector.tensor_copy(vts[sc * 4 + v], b[0])

                    units.append(evict)

                def qk_pass(w_sb, dst, d):
                    b = [None]

                    def mms(k, d=d):
                        if k == 0:
                            b[0] = bank(pidx[0] % 2)
                            pidx[0] += 1
                        for h in (2 * k, 2 * k + 1):
                            nc.tensor.matmul(
                                b[0], w_sb[:, h, d * 128:(d + 1) * 128],
                                st['xt'][h // XB][:, h % XB, :],
                                start=(h == 0), stop=(h == HC - 1))

                    for k in range(HC // 2):
                        units.append(lambda k=k, mms=mms: mms(k))

                    def evict(dst=dst):
                        # single PSUM read (copy), then RoPE in f16 SBUF —
                        # extra PSUM reads backpressure the PE drain port
                        t2 = work.tile([128, 512], MDT, tag="t2", bufs=3,
                                       name=f"t2_{sc}_{dst.tensor.name}")
                        nc.vector.tensor_copy(t2, b[0])     # frees the bank
                        t1 = work.tile([128, 512], MDT, tag="t1", bufs=3,
                                       name=f"t1_{sc}_{dst.tensor.name}")
                        nc.vector.tensor_mul(t1, t2, cos_sb[:, ssl])
                        nc.vector.tensor_mul(dst[0:64, :], t2[64:128, :],
                                             sin_sb[64:128, ssl])
                        nc.vector.tensor_mul(dst[64:128, :], t2[0:64, :],
                                             sin_sb[0:64, ssl])
                        nc.vector.tensor_add(dst, dst, t1)

                    units.append(evict)

                for v in range(4):
                    v_pass(v)
                for d in range(GH):
                    qk_pass(wq_sb, qts[d][sc], d)
                for d in range(GH):
                    qk_pass(wk_sb, kts[d][sc], d)
                return units

            # ---- B(qc): causal attention for one q chunk ---------------
            ctx_t = {}
            sps_idx = [0]
            head_idx = [0]

            def b_units(qc):
                if variant == "causal":
                    # diagonal blocks first (restricted to valid columns),
                    # then the full blocks below the diagonal
                    order = [(4 * qc + j, 128 * j) for j in range(4)] + \
                            [(kb, 0) for kb in range(4 * qc)]
                else:
                    order = [(kb, 0) for kb in range(HC)]
                units = []
                ctx_t[qc] = []
                for hd in range(GH):
                    st = {}

                    def start_head(st=st):
                        st['ctxps'] = bank(6 + head_idx[0] % 2)
                        head_idx[0] += 1
                        st['dacc'] = work.tile([128, 512], MDT, tag="dacc",
                                               bufs=2, name=f"dacc_{qc}_{hd}")
                        st['pend'] = None

                    def flush(last, st=st, hd=hd):
                        pexp, first, kbp, off = st['pend']
                        n = 512 - off
                        nc.tensor.matmul(st['ctxps'][:, off:512],
                                         vts[kbp][:, hd * 128:(hd + 1) * 128],
                                         pexp[:, 0:n], start=first, stop=last)

                    def kb_iter(i, kb, off, st=st, hd=hd,
                                start_head=start_head, flush=flush):
                        if i == 0:
                            start_head()
                        n = 512 - off
                        diag = variant == "causal" and kb >= 4 * qc
                        sps = bank(4 + sps_idx[0] % 2)
                        sps_idx[0] += 1
                        nc.tensor.matmul(
                            sps[:, off:512],
                            kts[hd][kb // 4][:, (kb % 4) * 128:(kb % 4 + 1) * 128],
                            qts[hd][qc][:, off:512], start=True, stop=not diag)
                        if diag:
                            # triangular mask on the 128-wide diagonal block
                            # as a second matmul into the same accumulation
                            # group: sum_c lower[c,k]*negdiag[c,t] =
                            # NEG2*[t<k] — keeps DVE out of the exp chain
                            nc.tensor.matmul(sps[:, off:off + 128], lind,
                                             negd, start=False, stop=True)
                        pexp = work.tile([128, 512], MDT, tag="pexp", bufs=4,
                                         name=f"pexp_{qc}_{hd}_{kb}")
                        # bias -4 keeps the f16 denominator accumulator well
                        # below overflow; it cancels in ctx/den
                        nc.scalar.activation(pexp[:, 0:n], sps[:, off:512],
                                             EXP, scale=SCALE, bias=ebias)
                        # denominator partial sum (DVE) is pended one
                        # iteration: it depends on exp(kb), and emitting it
                        # now would block the next mask-add (and so the next
                        # exp) behind it in the DVE FIFO
                        if st['pend'] is not None:
                            flush(False)
                            dadd(st)
                        st['pend'] = (pexp, i == 0, kb, off)

                    def dadd(st):
                        pexp, first, kbp, off = st['pend']
                        n = 512 - off
                        if first:
                            nc.vector.tensor_copy(st['dacc'], pexp)
                        else:
                            nc.vector.tensor_add(st['dacc'][:, off:512],
                                                 st['dacc'][:, off:512],
                                                 pexp[:, 0:n])

                    def tail(st=st, hd=hd, flush=flush):
                        flush(True)
                        dadd(st)
                        denps = bank(2, shape=(1, 512))
                        nc.tensor.matmul(denps, ones, st['dacc'],
                                         start=True, stop=True)
                        rcp = work.tile([1, 512], F32, tag="rcp", bufs=2,
                                        name=f"rcp_{qc}_{hd}")
                        nc.vector.reciprocal_approx_fast(rcp, denps)
                        dbc = work.tile([128, 512], F32, tag="dbc", bufs=2,
                                        name=f"dbc_{qc}_{hd}")
                        nc.gpsimd.partition_broadcast(dbc, rcp)
                        ct = work.tile([128, 512], MDT, tag="ctx", bufs=8,
                                       name=f"ctx_{qc}_{hd}")
                        nc.vector.tensor_mul(ct, st['ctxps'], dbc)  # frees ctx
                        ctx_t[qc].append(ct)

                    for i, (kb, off) in enumerate(order):
                        units.append(lambda i=i, kb=kb, off=off,
                                     kb_iter=kb_iter: kb_iter(i, kb, off))
                    units.append(tail)
                return units

            # ---- O(qc): o_proj for one q chunk -------------------------
            def o_units(qc, banks, mode="split"):
                units = []
                oidx = [0]

                ots = {}

                def oevict(ops, qb, ob, cnt, last=False):
                    # one wide ot tile per q block; a single [128,2048] DMA
                    # per q block keeps the sync queue's ~0.6us-per-DMA
                    # issue cost off the critical path. The very last block
                    # DMAs per-ob so the kernel doesn't end waiting on one
                    # long transfer.
                    if qb not in ots:
                        ots[qb] = work.tile([128, HID], IDT, tag="outsb",
                                            bufs=2, name=f"ot_{qc}_{qb}")
                    ot = ots[qb]
                    if mode == "split" and cnt % 2 == 0:
                        nc.scalar.copy(ot[:, ob * 512:(ob + 1) * 512], ops)
                    else:
                        nc.vector.tensor_copy(ot[:, ob * 512:(ob + 1) * 512],
                                              ops)
                    rows = slice((qc * 4 + qb) * 128, (qc * 4 + qb + 1) * 128)
                    if last:
                        nc.sync.dma_start(
                            out=out[rows, ob * 512:(ob + 1) * 512],
                            in_=ot[:, ob * 512:(ob + 1) * 512])
                    elif ob == 3:
                        nc.sync.dma_start(out=out[rows, :], in_=ot)

                def oproj(qb, ob):
                    ops = bank(banks[oidx[0] % len(banks)])
                    oidx[0] += 1
                    for hd in range(GH):
                        nc.tensor.matmul(
                            ops, ctx_t[qc][hd][:, qb * 128:(qb + 1) * 128],
                            wo_sb[:, hd, ob * 512:(ob + 1) * 512],
                            start=(hd == 0), stop=(hd == GH - 1))
                    oevict(ops, qb, ob, oidx[0])

                if len(banks) >= 4:
                    # 4-bank sets: hd-major with ob inner — each ct stationary
                    # is loaded once and streams 4 matmuls (no LDW
                    # serialization); sets alternate so evicts overlap
                    def oproj4(qb):
                        bset = banks[4 * (qb % (len(banks) // 4)):]
                        opss = [bank(bset[ob]) for ob in range(4)]
                        for hd in range(GH):
                            for ob in range(4):
                                nc.tensor.matmul(
                                    opss[ob],
                                    ctx_t[qc][hd][:, qb * 128:(qb + 1) * 128],
                                    wo_sb[:, hd, ob * 512:(ob + 1) * 512],
                                    start=(hd == 0), stop=(hd == GH - 1))
                        for ob in range(4):
                            oevict(opss[ob], qb, ob, ob, last=(qb == 3))

                    for qb in range(4):
                        units.append(lambda qb=qb, oproj4=oproj4: oproj4(qb))
                else:
                    for qb in range(4):
                        for ob in range(4):
                            units.append(lambda qb=qb, ob=ob, oproj=oproj:
                                         oproj(qb, ob))
                return units

            # ---- emit: P(0) | weight loads, then B(qc) | P(qc+1)+O(qc-1)
            p0 = p_units(0)
            wu = w_units()
            wi = 0
            for i, u in enumerate(p0):
                u()
                tgt = min(len(wu), (i + 1) * len(wu) * 3 // len(p0))
                while wi < tgt:
                    wu[wi]()
                    wi += 1
            while wi < len(wu):
                wu[wi]()
                wi += 1
            for qc in range(SC):
                bu = b_units(qc)
                fill = p_units(qc + 1) if qc + 1 < SC else []
                if qc >= 1:
                    # spread O(qc-1) units evenly through the fillers;
                    # in the last round ACT is paced by exp, so evict on DVE
                    ou = o_units(qc - 1, [3],
                             mode="dve" if qc == SC - 1 else "split")
                    merged = []
                    no, nf = len(ou), len(fill)
                    if nf == 0:
                        merged = ou
                    else:
                        oi = 0
                        for i, f in enumerate(fill):
                            merged.append(f)
                            tgt = (i + 1) * no // nf
                            while oi < tgt:
                                merged.append(ou[oi])
                                oi += 1
                        merged.extend(ou[oi:])
                    fill = merged
                na, nb = len(fill), len(bu)
                # reserve a few filler units past the round boundary so the
                # next round's softmax pipeline fill is hidden
                res = min(8, na)
                ai = 0
                for i, u in enumerate(bu):
                    u()
                    tgt = (i + 1) * (na - res) // nb
                    while ai < tgt:
                        fill[ai]()
                        ai += 1
                while ai < na:
                    fill[ai]()
                    ai += 1
            for u in o_units(SC - 1, [3, 0, 1, 4, 5, 6, 7, 2]):
                u()
    nc.compile()
    return nc


_CACHE = {}


def _get(variant, dt=None):
    dt = dt or DTYPE
    if (variant, dt) not in _CACHE:
        _CACHE[(variant, dt)] = _build(variant, dt)
    return _CACHE[(variant, dt)]


def _rope_tables():
    inv = 1.0 / (10000.0 ** (np.arange(0, D, 2, dtype=np.float64) / D))  # [64]
    t = np.arange(S, dtype=np.float64)
    fr = np.outer(inv, t)                       # [64, S]
    cosT = np.concatenate([np.cos(fr), np.cos(fr)], 0).astype(np.float32)
    # partition-swapped sign-folded sin: rows 0:64 = +sin, rows 64:128 = -sin
    sinT = np.concatenate([np.sin(fr), -np.sin(fr)], 0).astype(np.float32)
    return cosT, sinT


def _btpl_causal():
    # additive mask template: NEG where k > c-384 else 0
    k = np.arange(128)[:, None]
    c = np.arange(896)[None, :]
    return np.where(k > c - 384, np.float32(NEG), np.float32(0.0)).astype(np.float32)


def _np_cast(a, dt):
    if dt == "f16":
        return a.astype(np.float16)
    if dt == "bf16":
        import ml_dtypes
        return a.astype(ml_dtypes.bfloat16)
    return a


def _numpy_fallback(hs, Wq, Wk, Wv, Wo, mask):
    B = hs.shape[0]
    cosT, sinT = _rope_tables()
    cos = cosT.T[None, :, None, :]
    sin = np.abs(sinT).T[None, :, None, :]
    outs = []
    for b in range(B):
        x = hs[b]
        q = (x @ Wq).reshape(S, 16, D)[None]
        k = (x @ Wk).reshape(S, 16, D)[None]
        vv = (x @ Wv).reshape(S, 16, D)

        def rope(z):
            z1, z2 = z[..., :64], z[..., 64:]
            rot = np.concatenate([-z2, z1], -1)
            return z * cos + rot * sin

        q, k = rope(q)[0], rope(k)[0]
        o = np.empty((S, 16, D), np.float32)
        m = mask[0, 0]
        for h in range(16):
            sc = (q[:, h] @ k[:, h].T) * SCALE
            sc = np.where(m == 0, -np.inf, sc)
            sc -= sc.max(-1, keepdims=True)
            p = np.exp(sc)
            p /= p.sum(-1, keepdims=True)
            o[:, h] = p @ vv[:, h]
        outs.append(o.reshape(S, HID) @ Wo)
    return np.stack(outs).astype(np.float32)


def _tile_xt(hsT, dt):
    # [2048 h, 2048 s] -> [128 p, 4 sc, 4 j, 4 hh, 512] with h = (4j+hh)*128+p
    a = _np_cast(hsT, dt).reshape(HC, 128, SC, 512)
    a = a.transpose(1, 2, 0, 3).reshape(128, SC, NXT_H, 4, 512)
    return np.ascontiguousarray(a)


def _tile_xt0(hsT, dt):
    # sc=0 slice in v-block-major layout: [128 p, 4 v, 4 j, 4 hh, 128]
    a = _np_cast(hsT[:, 0:512], dt).reshape(HC, 128, 4, 128)
    a = a.transpose(1, 2, 0, 3).reshape(128, 4, NXT_H, 4, 128)
    return np.ascontiguousarray(a)


NXT_H = HC // 4


def _tile_w(w, dt):
    # [2048 h, 512] -> [128 p, 16 c, 512] with h = c*128+p
    a = _np_cast(w, dt).reshape(HC, 128, GW).transpose(1, 0, 2)
    return np.ascontiguousarray(a)


def _tile_wo(w, dt):
    # [512 r, 2048] -> [128 p, 4 hd, 2048] with r = hd*128+p
    a = _np_cast(w, dt).reshape(GH, 128, HID).transpose(1, 0, 2)
    return np.ascontiguousarray(a)


def make_in_maps(inputs, variant):
    hs = np.asarray(inputs["hidden_states"], dtype=np.float32)
    Wq, Wk, Wv, Wo = (np.asarray(inputs[w], dtype=np.float32)
                      for w in ("Wq", "Wk", "Wv", "Wo"))
    cosT, sinT = _rope_tables()
    btpl = _btpl_causal() if variant == "causal" else np.zeros((128, 896), np.float32)

    in_maps = []
    for c in range(NCORES):
        b, g = divmod(c, GH)
        gsl = slice(g * GW, (g + 1) * GW)
        hsT = np.ascontiguousarray(hs[b].T)
        in_maps.append({
            "xt": _tile_xt(hsT, DTYPE),
            "xt0": _tile_xt0(hsT, DTYPE),
            "wq": _tile_w(Wq[:, gsl], DTYPE),
            "wk": _tile_w(Wk[:, gsl], DTYPE),
            "wv": _tile_w(Wv[:, gsl], DTYPE),
            "wo": _tile_wo(Wo[gsl, :], DTYPE),
            "cost": _np_cast(cosT, DTYPE), "sint": _np_cast(sinT, DTYPE),
            "btpl": btpl,
            "lindt": _np_cast(np.triu(np.ones((128, 128), np.float32), 1),
                              DTYPE),
            "negdt": _np_cast(np.diag(np.full(128, -60000.0,
                                              np.float32)), DTYPE),
        })
    return in_maps


def kernel(hidden_states, Wq, Wk, Wv, Wo, attention_mask):
    hs = np.asarray(hidden_states, dtype=np.float32)
    Wq, Wk, Wv, Wo = (np.asarray(w, dtype=np.float32) for w in (Wq, Wk, Wv, Wo))
    mask = np.asarray(attention_mask)
    B = hs.shape[0]

    m3 = mask.reshape(-1, mask.shape[-2], mask.shape[-1])
    m2 = m3[0]
    same = all(np.array_equal(m2, m3[i]) for i in range(1, m3.shape[0]))
    if not same:
        return _numpy_fallback(hs, Wq, Wk, Wv, Wo, mask)
    if np.all(m2 == 1):
        variant = "full"
    elif np.array_equal(m2 != 0, np.tril(np.ones((S, S), dtype=bool))):
        variant = "causal"
    else:
        return _numpy_fallback(hs, Wq, Wk, Wv, Wo, mask)

    in_maps = make_in_maps(
        {"hidden_states": hs, "Wq": Wq, "Wk": Wk, "Wv": Wv, "Wo": Wo}, variant)

    nc = _get(variant)
    res = run_bass_kernel_spmd(nc, in_maps, list(range(NCORES))).results
    out = np.zeros((B, S, HID), np.float32)
    for c in range(NCORES):
        b = c // GH
        out[b] += res[c]["out"]
    return out
